# revision 20
# baseline (speedup 1.0000x reference)
"""AlternateGCN on 8 TRN2 NeuronCores.

Strategy (edge/data parallel, dest-sharded):
  - Edges sharded by DESTINATION-node owner core (8 node shards of 6272),
    sorted by destination. Segment-sum runs ON DEVICE as dense PE matmuls
    in TRANSPOSED orientation: per 128-edge tile, Y^T[d, n] += M^T @ St
    accumulates in PSUM, where M = per-edge message tile (fp16 stream,
    GCN normalization c_e = dinv[row]*w*dinv[col] pre-folded during the
    host-side gather, like PyG's cached gcn_norm) and St[e, n] is an EXACT
    0/1 selection matrix streamed in fp8e4 (halves its DMA bytes; 0/1 are
    exact in fp8). No per-tile DVE build ops -> no DVE-sequencer
    bottleneck; conv NEFFs are DMA-roofline bound.
  - ELU computed as elu(z)+1 = relu(z) + exp(-relu(-z)); the "-1" is
    folded into the NEXT layer's bias; 3 ACT ops + 1 DVE op per chunk.
  - Degree -> dinv still computed on device (N1); host only gathers
    dinv[row]*w*dinv[col] into the message stream during shard/sort prep.
  - Decoder (N4): streams u[src], v[dst] as fp16 [feat, edge] streams;
    u+v is computed BY THE DMA (SWDGE accumulate-DMA of the v stream onto
    the u tile), relu on DVE, W2 matmul on PE, relu+bias alternating
    ACT/DVE, and the W3 dot-product uses a sliding-window diagonal
    stationary (w3 placed at a chunk-dependent column of a zero tile) so
    16 chunks' scalar outputs land on 16 distinct PSUM partitions and are
    copied out in ONE [16, 512] op -- no single-partition copies.
  - 4 NEFF stages: N1 degrees->dinv; N2 layer-1 conv -> h2 (pre-multiplied
    by W_out); N3 layer-2 conv -> uT,vT decoder tables; N4 edge MLP.

Self-contained; hardcodes problem shapes (N=50000, E=800000, D=128, H=256).
"""
import sys, time
sys.path.insert(0, "/opt/trn_rl_repo")
import numpy as np
import jax
from jax.sharding import Mesh, PartitionSpec
from jax.experimental.shard_map import shard_map

import concourse.bacc as bacc
import concourse.bass as bass
import concourse.mybir as mybir
import concourse.tile as tile
from concourse import bass2jax
from concourse.bass2jax import _bass_exec_p, install_neuronx_cc_hook

P = 128
NCORES = 8
AF = mybir.ActivationFunctionType
ALU = mybir.AluOpType
DT = mybir.dt
F8 = DT.float8e4
F16 = DT.float16
F32 = DT.float32
NPF8 = DT.np(F8)
VERBOSE = bool(int(__import__("os").environ.get("KERNEL_VERBOSE", "0")))


def _log(msg):
    if VERBOSE:
        print(f"[kernel {time.strftime('%H:%M:%S')}] {msg}", flush=True)


# ---------------------------------------------------------------- runner ----
class SpmdRunner:
    def __init__(self, nc, n_cores=NCORES, donate=False):
        install_neuronx_cc_hook()
        self.nc, self.n_cores = nc, n_cores
        self.donate = donate
        pname = nc.partition_id_tensor.name if nc.partition_id_tensor else None
        in_names, out_names, out_avals, zero_outs = [], [], [], []
        for alloc in nc.m.functions[0].allocations:
            if not isinstance(alloc, mybir.MemoryLocationSet):
                continue
            name = alloc.memorylocations[0].name
            if alloc.kind == "ExternalInput":
                if name != pname:
                    in_names.append(name)
            elif alloc.kind == "ExternalOutput":
                out_names.append(name)
                shape = tuple(alloc.tensor_shape)
                dtype = DT.np(alloc.dtype)
                out_avals.append(jax.core.ShapedArray(shape, dtype))
                zero_outs.append(np.zeros(shape, dtype))
        self.in_names, self.out_names = in_names, out_names
        self.out_avals, self.zero_outs = out_avals, zero_outs
        self.n_params, self.n_outs = len(in_names), len(out_avals)
        all_in = list(in_names) + list(out_names)
        if pname is not None:
            all_in.append(pname)

        def _body(*args):
            operands = list(args)
            if pname is not None:
                operands.append(bass2jax.partition_id_tensor())
            return tuple(_bass_exec_p.bind(
                *operands, out_avals=tuple(out_avals), in_names=tuple(all_in),
                out_names=tuple(out_names), lowering_input_output_aliases=(),
                sim_require_finite=True, sim_require_nnan=True, nc=nc))

        devices = jax.devices()[:n_cores]
        mesh = Mesh(np.asarray(devices), ("core",))
        donate_idx = tuple(range(self.n_params, self.n_params + self.n_outs)) \
            if self.donate else ()
        self.fn = jax.jit(
            shard_map(_body, mesh=mesh,
                      in_specs=(PartitionSpec("core"),) * (self.n_params + self.n_outs),
                      out_specs=(PartitionSpec("core"),) * len(out_names)),
            donate_argnums=donate_idx, keep_unused=True)
        self.dev_inputs = None
        self.dev_zeros = None

    def stage(self, in_maps):
        concat = [np.concatenate([np.ascontiguousarray(in_maps[c][n])
                                  for c in range(self.n_cores)], axis=0)
                  for n in self.in_names]
        self.dev_inputs = [jax.device_put(a) for a in concat]
        jax.block_until_ready(self.dev_inputs)

    def run(self):
        if self.donate:
            zeros = [np.zeros((self.n_cores * z.shape[0], *z.shape[1:]), z.dtype)
                     for z in self.zero_outs]
        else:
            if self.dev_zeros is None:
                self.dev_zeros = [jax.device_put(
                    np.zeros((self.n_cores * z.shape[0], *z.shape[1:]), z.dtype))
                    for z in self.zero_outs]
                jax.block_until_ready(self.dev_zeros)
            zeros = self.dev_zeros
        out = self.fn(*self.dev_inputs, *zeros)
        jax.block_until_ready(out)
        return out

    def results(self, out_arrs):
        return [
            {n: np.asarray(out_arrs[i]).reshape(self.n_cores, *self.out_avals[i].shape)[c]
             for i, n in enumerate(self.out_names)}
            for c in range(self.n_cores)]


# ------------------------------------------------------------- geometry ----
class Cfg:
    def __init__(self, N=50000, D=128, H=256):
        self.N, self.D, self.H = N, D, H
        self.NPAD = ((N + NCORES * P - 1) // (NCORES * P)) * (NCORES * P)
        self.SHARD = self.NPAD // NCORES
        self.WPC = self.SHARD // P           # real windows per core (49)
        self.WGL = self.NPAD // P            # global windows
        self.BW = 4                          # windows per epilogue block
        self.WSLOT = ((self.WPC + self.BW - 1) // self.BW) * self.BW  # 52


# ------------------------------------------------------- device helpers ----
def _rsqrt_masked(nc, out_sb, deg_sb, pool, shape):
    """out = 1/sqrt(deg) where deg>0 else 0 (no Rsqrt, no inf*0)."""
    m = pool.tile(shape, F32, tag="rq_m")
    t = pool.tile(shape, F32, tag="rq_t")
    nc.vector.tensor_scalar(out=m[:], in0=deg_sb, scalar1=0.0, scalar2=None,
                            op0=ALU.is_gt)
    nc.scalar.activation(out=t[:], in_=deg_sb, func=AF.Sqrt)
    nc.vector.tensor_scalar(out=t[:], in0=t[:], scalar1=1.0, scalar2=None,
                            op0=ALU.add)
    nc.vector.tensor_tensor(out=t[:], in0=t[:], in1=m[:], op=ALU.subtract)
    nc.vector.reciprocal(out=out_sb, in_=t[:])
    nc.vector.tensor_tensor(out=out_sb, in0=out_sb, in1=m[:], op=ALU.mult)


# ------------------------------------------------------------------ N1 -----
def build_n1(cfg, K1, K2, repeat=1):
    """Per-shard degree reduce for both graphs -> dinv1, dinv2 [128, WPC]."""
    nc = bacc.Bacc("TRN2", target_bir_lowering=False)
    W = cfg.WPC
    wp1 = nc.dram_tensor("wp1", [P, W * K1], F32, kind="ExternalInput")
    wp2 = nc.dram_tensor("wp2", [P, W * K2], F32, kind="ExternalInput")
    dinv1 = nc.dram_tensor("dinv1", [P, W], F32, kind="ExternalOutput")
    dinv2 = nc.dram_tensor("dinv2", [P, W], F32, kind="ExternalOutput")
    with tile.TileContext(nc) as tc:
        with tc.tile_pool(name="p", bufs=1) as pool:
            for _ in range(repeat):
                for tag, wp, K, dout in (("a", wp1, K1, dinv1),
                                         ("b", wp2, K2, dinv2)):
                    wt = pool.tile([P, W, K], F32, tag=f"wt{tag}")
                    nc.sync.dma_start(out=wt[:].rearrange("p a b -> p (a b)"),
                                      in_=wp[:, :])
                    deg = pool.tile([P, W], F32, tag=f"deg{tag}")
                    nc.vector.tensor_reduce(out=deg[:, :, None], in_=wt[:],
                                            axis=mybir.AxisListType.X, op=ALU.add)
                    o = pool.tile([P, W], F32, tag=f"o{tag}")
                    _rsqrt_masked(nc, o[:], deg[:], pool, [P, W])
                    nc.sync.dma_start(out=dout[:, :], in_=o[:])
    nc.finalize()
    return nc


# ----------------------------------------------------------- N2 and N3 -----
def build_conv(cfg, bt, layer, repeat=1):
    """Conv layer, transposed pipeline.

    bt: per-slot tile budgets [WSLOT]. T = sum(bt).
    Streams: m1 [P, T*D] fp16 = c_e * x[row_e] messages; st8 [P, T*P]
    fp8e4 EXACT 0/1 selection (st8[e, n] = 1 iff col_rel_e == n).
    Per tile: one matmul Y^T += m1_tile^T @ st_tile.
    layer=1: x1T = elu(W_in^T @ Y1T + b_in) -> h2T = W_out^T @ x1T -> h2s.
    layer=2: x2T = elu(Y2T + b_out) -> uT = W1a^T x2T + b1, vT = W1b^T x2T.
    """
    nc = bacc.Bacc("TRN2", target_bir_lowering=False)
    D, H, BW, WSLOT = cfg.D, cfg.H, cfg.BW, cfg.WSLOT
    T = int(np.sum(bt))
    to = np.zeros(WSLOT + 1, np.int64)
    np.cumsum(bt, out=to[1:])
    BTMAX = int(np.max(bt))
    NBLK = WSLOT // BW
    NB = BW * P                              # block node columns (512)

    m1 = nc.dram_tensor("m1", [P, T * D], F16, kind="ExternalInput")
    st8 = nc.dram_tensor("st8", [P, T * P], F8, kind="ExternalInput")
    if layer == 1:
        w_in = nc.dram_tensor("w_in", [D, H], F16, kind="ExternalInput")
        b_in = nc.dram_tensor("b_in", [P, 2], F32, kind="ExternalInput")
        w_out = nc.dram_tensor("w_out", [H, D], F16, kind="ExternalInput")
        h2s = nc.dram_tensor("h2s", [P, WSLOT * P], F16, kind="ExternalOutput")
    else:
        w1 = nc.dram_tensor("w1", [2 * D, D], F16, kind="ExternalInput")
        b1v = nc.dram_tensor("b1v", [P, 1], F32, kind="ExternalInput")
        b_out = nc.dram_tensor("b_out", [P, 1], F32, kind="ExternalInput")
        us = nc.dram_tensor("us", [P, WSLOT * P], F16, kind="ExternalOutput")
        vs = nc.dram_tensor("vs", [P, WSLOT * P], F16, kind="ExternalOutput")

    with tile.TileContext(nc) as tc:
        with tc.tile_pool(name="const", bufs=1) as cst:
            if layer == 1:
                w_in_sb = cst.tile([P, H], F16, tag="w_in")
                nc.sync.dma_start(out=w_in_sb[:], in_=w_in[:, :])
                wo_a = cst.tile([P, D], F16, tag="wo_a")
                wo_b = cst.tile([P, D], F16, tag="wo_b")
                nc.sync.dma_start(out=wo_a[:], in_=w_out[0:P, :])
                nc.sync.dma_start(out=wo_b[:], in_=w_out[P:2 * P, :])
                b_in_sb = cst.tile([P, 2], F32, tag="b_in")
                nc.sync.dma_start(out=b_in_sb[:], in_=b_in[:, :])
                nb_in = cst.tile([P, 2], F32, tag="nb_in")
                nc.vector.tensor_scalar(out=nb_in[:], in0=b_in_sb[:],
                                        scalar1=-1.0, scalar2=None, op0=ALU.mult)
            else:
                w1a = cst.tile([P, D], F16, tag="w1a")
                w1b = cst.tile([P, D], F16, tag="w1b")
                nc.sync.dma_start(out=w1a[:], in_=w1[0:P, :])
                nc.sync.dma_start(out=w1b[:], in_=w1[P:2 * P, :])
                b1_sb = cst.tile([P, 1], F32, tag="b1")
                nc.sync.dma_start(out=b1_sb[:], in_=b1v[:, :])
                bo_sb = cst.tile([P, 1], F32, tag="bo")
                nc.sync.dma_start(out=bo_sb[:], in_=b_out[:, :])
                nbo_sb = cst.tile([P, 1], F32, tag="nbo")
                nc.vector.tensor_scalar(out=nbo_sb[:], in0=bo_sb[:],
                                        scalar1=-1.0, scalar2=None, op0=ALU.mult)
            _conv_main(nc, tc, cfg, bt, layer, repeat, locals())
    nc.finalize()
    return nc


def _conv_main(nc, tc, cfg, bt, layer, repeat, env):
    D, H, BW, WSLOT = cfg.D, cfg.H, cfg.BW, cfg.WSLOT
    to = np.zeros(WSLOT + 1, np.int64)
    np.cumsum(bt, out=to[1:])
    BTMAX = int(np.max(bt))
    NBLK = WSLOT // BW
    NB = BW * P
    m1, st8 = env["m1"], env["st8"]
    if layer == 1:
        w_in_sb, wo_a, wo_b = env["w_in_sb"], env["wo_a"], env["wo_b"]
        b_in_sb, nb_in = env["b_in_sb"], env["nb_in"]
        h2s = env["h2s"]
    else:
        w1a, w1b = env["w1a"], env["w1b"]
        bo_sb, nbo_sb = env["bo_sb"], env["nbo_sb"]
        b1_sb = env["b1_sb"]
        us, vs = env["us"], env["vs"]
    # DMA super-blocks of 8 windows (~6 MB transfers) for near-roofline DMA;
    # epilogue still runs per 4-window block.
    SBW = 2 * BW
    NSUP = (WSLOT + SBW - 1) // SBW
    sup_lo = [b * SBW for b in range(NSUP)]
    sup_hi = [min(WSLOT, b * SBW + SBW) for b in range(NSUP)]
    MSUP = int(max(to[h] - to[l] for l, h in zip(sup_lo, sup_hi)))
    with (tc.tile_pool(name="mt", bufs=3) as mtp,
          tc.tile_pool(name="stt", bufs=3) as stp,
          tc.tile_pool(name="ypsum", bufs=3, space="PSUM") as yp,
          tc.tile_pool(name="xpsum", bufs=2, space="PSUM") as xp,
          tc.tile_pool(name="hpsum", bufs=2, space="PSUM") as hp,
          tc.tile_pool(name="slab", bufs=2) as slp,
          tc.tile_pool(name="fl", bufs=2) as fl):
            for _ in range(repeat):
                for blk in range(NBLK):
                    sup = (blk * BW) // SBW
                    if blk * BW == sup_lo[sup]:
                        b0 = int(to[sup_lo[sup]])
                        nb = int(to[sup_hi[sup]]) - b0
                        mw = mtp.tile([P, MSUP * D], F16, tag="mw")
                        nc.sync.dma_start(out=mw[:, 0:nb * D],
                                          in_=m1[:, b0 * D:(b0 + nb) * D])
                        sw = stp.tile([P, MSUP * P], F8, tag="sw")
                        nc.scalar.dma_start(out=sw[:, 0:nb * P],
                                            in_=st8[:, b0 * P:(b0 + nb) * P])
                    ysl = slp.tile([P, NB], F16, tag="ysl")
                    for j in range(BW):
                        k = blk * BW + j
                        nt = int(bt[k])
                        lo = int(to[k]) - b0
                        y = yp.tile([P, P], F32, tag="y")
                        for s in range(nt):
                            t = lo + s
                            nc.tensor.matmul(out=y[:],
                                             lhsT=mw[:, t * D:(t + 1) * D],
                                             rhs=sw[:, t * P:(t + 1) * P],
                                             start=(s == 0), stop=(s == nt - 1))
                        # PSUM -> SBUF copy on DVE (ACT is epilogue-bound)
                        nc.vector.tensor_scalar(out=ysl[:, j * P:(j + 1) * P],
                                                in0=y[:], scalar1=0.0,
                                                scalar2=None, op0=ALU.add)
                    if layer == 1:
                        x1c = []
                        for hh in range(2):
                            x1p = xp.tile([P, NB], F32, tag="x1p")
                            nc.tensor.matmul(
                                out=x1p[:],
                                lhsT=w_in_sb[:, hh * P:(hh + 1) * P],
                                rhs=ysl[:], start=True, stop=True)
                            # elu(z) = (relu(z) - 1) + exp(-relu(-z));
                            # a/e kept f32 so the near-1 cancellation is exact
                            a = fl.tile([P, NB], F32, tag=f"a{hh}")
                            nc.vector.tensor_scalar(
                                out=a[:], in0=x1p[:],
                                scalar1=b_in_sb[:, hh:hh + 1], scalar2=0.0,
                                op0=ALU.add, op1=ALU.max)
                            rn = fl.tile([P, NB], F16, tag=f"rn{hh}")
                            nc.scalar.activation(out=rn[:], in_=x1p[:],
                                                 func=AF.Relu, scale=-1.0,
                                                 bias=nb_in[:, hh:hh + 1])
                            e = fl.tile([P, NB], F32, tag=f"e{hh}")
                            nc.scalar.activation(out=e[:], in_=rn[:],
                                                 func=AF.Exp, scale=-1.0)
                            x1 = fl.tile([P, NB], F16, tag=f"x1{hh}")
                            nc.vector.scalar_tensor_tensor(
                                out=x1[:], in0=a[:], scalar=-1.0, in1=e[:],
                                op0=ALU.add, op1=ALU.add)
                            x1c.append(x1)
                        h2p = hp.tile([P, NB], F32, tag="h2p")
                        nc.tensor.matmul(out=h2p[:], lhsT=wo_a[:],
                                         rhs=x1c[0][:], start=True, stop=False)
                        nc.tensor.matmul(out=h2p[:], lhsT=wo_b[:],
                                         rhs=x1c[1][:], start=False, stop=True)
                        hsb = fl.tile([P, NB], F16, tag="hsb")
                        nc.scalar.activation(out=hsb[:], in_=h2p[:],
                                             func=AF.Identity)
                        nc.sync.dma_start(
                            out=h2s[:, blk * NB:(blk + 1) * NB], in_=hsb[:])
                    else:
                        # x2 = elu(ysl + b_out), same f32-cancellation scheme
                        a = fl.tile([P, NB], F32, tag="a2")
                        nc.scalar.activation(out=a[:], in_=ysl[:],
                                             func=AF.Relu, bias=bo_sb[:, 0:1])
                        rn = fl.tile([P, NB], F16, tag="rn2")
                        nc.scalar.activation(out=rn[:], in_=ysl[:],
                                             func=AF.Relu, scale=-1.0,
                                             bias=nbo_sb[:, 0:1])
                        e = fl.tile([P, NB], F32, tag="e2")
                        nc.scalar.activation(out=e[:], in_=rn[:],
                                             func=AF.Exp, scale=-1.0)
                        x2 = fl.tile([P, NB], F16, tag="x2")
                        nc.vector.scalar_tensor_tensor(
                            out=x2[:], in0=a[:], scalar=-1.0, in1=e[:],
                            op0=ALU.add, op1=ALU.add)
                        up = xp.tile([P, NB], F32, tag="up")
                        nc.tensor.matmul(out=up[:], lhsT=w1a[:], rhs=x2[:],
                                         start=True, stop=True)
                        u_sb = fl.tile([P, NB], F16, tag="u_sb")
                        nc.scalar.activation(out=u_sb[:], in_=up[:],
                                             func=AF.Identity,
                                             bias=b1_sb[:, 0:1])
                        nc.sync.dma_start(
                            out=us[:, blk * NB:(blk + 1) * NB], in_=u_sb[:])
                        vp = hp.tile([P, NB], F32, tag="vp")
                        nc.tensor.matmul(out=vp[:], lhsT=w1b[:], rhs=x2[:],
                                         start=True, stop=True)
                        v_sb = fl.tile([P, NB], F16, tag="v_sb")
                        nc.scalar.activation(out=v_sb[:], in_=vp[:],
                                             func=AF.Identity)
                        nc.sync.dma_start(
                            out=vs[:, blk * NB:(blk + 1) * NB], in_=v_sb[:])


# ------------------------------------------------------------------ N4 -----
N4_CH = 512                                  # edges per chunk (1 PSUM bank)
N4_CB = 16                                   # chunks per block / out rows
N4_BLK = N4_CH * N4_CB                       # 8192 edges per block


def build_n4(cfg, S4, repeat=1):
    """Decoder: out = relu(relu(u[src]+v[dst]) @ W2 + b2) @ W3 + b3.

    guv: single host-interleaved fp16 stream ([u-block | v-block] per
    8192-edge block -> one 4 MB DMA per block). Per 1024-edge sub-chunk:
    u+v and relu on DVE, two W2 matmuls into a 2-bank PSUM tile, one
    relu+bias on ACT, and W3 matmuls whose stationary is a sliding window
    of a zero tile with w3 at column N4_CB, so chunk c's [1, 512] result
    lands on PSUM partition c; one [16, 512] bias-copy per block drains
    all 16 chunks. Output: oute[c, g*512+col] = edge g*8192+c*512+col.
    (A SWDGE accumulate-DMA for u+v desyncs the axon mesh - done on DVE.)
    """
    nc = bacc.Bacc("TRN2", target_bir_lowering=False)
    D = cfg.D
    CH, CB, BLK = N4_CH, N4_CB, N4_BLK
    NBLK = S4 // BLK
    assert S4 % BLK == 0
    guv = nc.dram_tensor("guv", [P, 2 * S4], F16, kind="ExternalInput")
    w2 = nc.dram_tensor("w2", [D, D], F16, kind="ExternalInput")
    b2v = nc.dram_tensor("b2v", [P, 1], F32, kind="ExternalInput")
    w3z = nc.dram_tensor("w3z", [P, 2 * CB], F16, kind="ExternalInput")
    b3v = nc.dram_tensor("b3v", [CB, 1], F32, kind="ExternalInput")
    oute = nc.dram_tensor("oute", [CB, NBLK * CH], F16, kind="ExternalOutput")
    with tile.TileContext(nc) as tc:
        with (tc.tile_pool(name="const", bufs=1) as cst,
              tc.tile_pool(name="guv", bufs=4) as gup,
              tc.tile_pool(name="h1r", bufs=3) as hrp,
              tc.tile_pool(name="h2ps", bufs=3, space="PSUM") as h2ps,
              tc.tile_pool(name="ops", bufs=2, space="PSUM") as ops,
              tc.tile_pool(name="sb", bufs=4) as sbp,
              tc.tile_pool(name="ob", bufs=2) as obp):
            w2_sb = cst.tile([P, D], F16, tag="w2")
            nc.sync.dma_start(out=w2_sb[:], in_=w2[:, :])
            w3z_sb = cst.tile([P, 2 * CB], F16, tag="w3z")
            nc.sync.dma_start(out=w3z_sb[:], in_=w3z[:, :])
            b2_sb = cst.tile([P, 1], F32, tag="b2")
            nc.sync.dma_start(out=b2_sb[:], in_=b2v[:, :])
            b3_sb = cst.tile([CB, 1], F32, tag="b3")
            nc.sync.dma_start(out=b3_sb[:], in_=b3v[:, :])
            for _ in range(repeat):
                for blk in range(NBLK):
                    guvt = gup.tile([P, 2 * BLK], F16, tag="guv")
                    nc.sync.dma_start(out=guvt[:],
                                      in_=guv[:, blk * 2 * BLK:(blk + 1) * 2 * BLK])
                    op = ops.tile([CB, CH], F32, tag="op")
                    for c2 in range(CB // 2):
                        # 1024-edge sub-chunks: halves ACT/DVE op overheads
                        dsl = slice(c2 * 2 * CH, (c2 + 1) * 2 * CH)
                        vsl = slice(BLK + c2 * 2 * CH, BLK + (c2 + 1) * 2 * CH)
                        h1a = hrp.tile([P, 2 * CH], F16, tag="h1a")
                        nc.vector.tensor_tensor(out=h1a[:], in0=guvt[:, dsl],
                                                in1=guvt[:, vsl], op=ALU.add)
                        h1r = hrp.tile([P, 2 * CH], F16, tag="h1r")
                        nc.vector.tensor_scalar(out=h1r[:], in0=h1a[:],
                                                scalar1=0.0, scalar2=None,
                                                op0=ALU.max)
                        h2 = h2ps.tile([P, 2 * CH], F32, tag="h2")
                        nc.tensor.matmul(out=h2[:, 0:CH], lhsT=w2_sb[:],
                                         rhs=h1r[:, 0:CH],
                                         start=True, stop=True)
                        nc.tensor.matmul(out=h2[:, CH:2 * CH], lhsT=w2_sb[:],
                                         rhs=h1r[:, CH:2 * CH],
                                         start=True, stop=True)
                        h2r = sbp.tile([P, 2 * CH], F16, tag="h2r")
                        nc.scalar.activation(out=h2r[:], in_=h2[:],
                                             func=AF.Relu,
                                             bias=b2_sb[:, 0:1])
                        for ci in range(2):
                            c = 2 * c2 + ci
                            # chunk c's dot-product lands on PSUM row c:
                            # lhsT = w3z[:, CB-c : 2*CB-c] has w3 at column c
                            nc.tensor.matmul(
                                out=op[:],
                                lhsT=w3z_sb[:, CB - c:2 * CB - c],
                                rhs=h2r[:, ci * CH:(ci + 1) * CH],
                                start=(c == 0), stop=(c == CB - 1))
                    otb = obp.tile([CB, CH], F16, tag="otb")
                    nc.scalar.activation(out=otb[:], in_=op[:],
                                         func=AF.Identity, bias=b3_sb[:, 0:1])
                    nc.sync.dma_start(
                        out=oute[:, blk * CH:(blk + 1) * CH], in_=otb[:])
    nc.finalize()
    return nc


# ------------------------------------------------------------ host prep ----
def _shard_graph(cfg, ei, w=None):
    """Shard edges by col-owner core, sort by col, assign windows to slots
    sorted by edge count so a shared per-slot budget has little padding."""
    row, col = np.asarray(ei[0]), np.asarray(ei[1])
    owner = col // cfg.SHARD
    cores = []
    for c in range(NCORES):
        sel = np.where(owner == c)[0]
        o = np.argsort(col[sel], kind="stable")
        sel = sel[o]
        colc = col[sel]
        win = (colc - c * cfg.SHARD) // P
        cnt = np.bincount(win, minlength=cfg.WPC)
        perm = np.argsort(-cnt, kind="stable")
        cores.append((sel, colc, win, cnt, perm))
    tiles = np.ones((NCORES, cfg.WSLOT), np.int64)
    for c in range(NCORES):
        _, _, _, cnt, perm = cores[c]
        tiles[c, :cfg.WPC] = np.maximum((cnt[perm] + P - 1) // P, 1)
    bt = tiles.max(axis=0)
    to = np.zeros(cfg.WSLOT + 1, np.int64)
    np.cumsum(bt, out=to[1:])
    T = int(to[-1])
    S = T * P
    out = []
    for c in range(NCORES):
        sel, colc, win, cnt, perm = cores[c]
        offs = np.zeros(cfg.WPC + 1, np.int64)
        np.cumsum(cnt, out=offs[1:])
        pos = np.arange(len(sel)) - offs[win]
        islot = np.empty(cfg.WPC, np.int64)
        islot[perm] = np.arange(cfg.WPC)
        slot = to[islot[win]] * P + pos
        rows_s = np.zeros(S, np.int64)
        cols_s = np.zeros(S, np.int64)
        colrel = np.zeros(S, np.int64)
        valid = np.zeros(S, bool)
        rows_s[slot] = row[sel]
        cols_s[slot] = colc
        colrel[slot] = colc % P
        valid[slot] = True
        d = dict(sel=sel, slot=slot, rows=rows_s, cols=cols_s, colrel=colrel,
                 valid=valid, perm=perm)
        if w is not None:
            ws = np.zeros(S, np.float32)
            ws[slot] = w[sel]
            d["wv"] = ws
        out.append(d)
    return out, bt, T


def _feat_stream(tbl, rows, coef, T, D):
    """[T*128] row ids + per-edge coefficients -> [128, T*D] fp16 message
    stream (coefficient-scaled gather)."""
    g = tbl[rows.reshape(T, P)].astype(np.float32)          # [T, 128, D]
    g *= coef.reshape(T, P)[:, :, None]
    return np.ascontiguousarray(
        g.transpose(1, 0, 2).reshape(P, T * D)).astype(np.float16)


def _sel_stream(colrel, valid, T):
    """[T*128] col_rel + valid -> [128, T*128] fp8 EXACT 0/1 selection."""
    st = np.zeros((P, T * P), NPF8)
    slot = np.where(valid)[0]
    p = slot % P
    t = slot // P
    st[p, t * P + colrel[slot]] = 1.0
    return st


def _unperm(cfg, shards, key):
    """Reassemble [128, NPAD] canonical node-major-T table from per-core
    permuted slot outputs [128, WSLOT*128]."""
    full = np.zeros((P, cfg.NPAD), np.float16)
    cols = np.arange(P)
    for c in range(NCORES):
        perm = shards[c]["perm"]
        src = shards[c][key]
        idx = (c * cfg.SHARD + (perm[:, None] * P + cols[None, :])).reshape(-1)
        full[:, idx] = src[:, :cfg.WPC * P]
    return full


def _degpad(cfg, col, w):
    """Per-core [128, WPC*K] padded weight layout for on-device degree sum."""
    o = np.argsort(col, kind="stable")
    cs, ws = col[o], w[o]
    cnt = np.bincount(cs, minlength=cfg.NPAD)
    K = int(max(cnt.max(), 1))
    offs = np.zeros(cfg.NPAD + 1, np.int64)
    np.cumsum(cnt, out=offs[1:])
    slotn = np.arange(len(cs)) - offs[cs]
    wpad = np.zeros((cfg.NPAD, K), np.float32)
    wpad[cs, slotn] = ws
    per_core = []
    for c in range(NCORES):
        blockc = wpad[c * cfg.SHARD:(c + 1) * cfg.SHARD]
        per_core.append(np.ascontiguousarray(
            blockc.reshape(cfg.WPC, P, K).transpose(1, 0, 2)
            .reshape(P, cfg.WPC * K)))
    return per_core, K


# --------------------------------------------------------------- kernel ----
_TIMES = {}
_DBG = {}


def kernel(node_ids, edge_index, neighbour_edge_index, edge_attr,
           emb, W_in, b_in, W_out, b_out, W1, b1, W2, b2, W3, b3):
    cfg = Cfg()
    D, H = cfg.D, cfg.H
    t_all = time.time()
    node_ids = np.asarray(node_ids)
    emb = np.asarray(emb, np.float32)
    edge_attr = np.asarray(edge_attr, np.float32)
    ei1 = np.asarray(edge_index)
    ei2 = np.asarray(neighbour_edge_index)
    E = ei1.shape[1]
    x_pad = np.zeros((cfg.NPAD, D), np.float32)
    x_pad[:cfg.N] = emb[node_ids]

    # ---- host index prep -------------------------------------------------
    e1, bt1, T1 = _shard_graph(cfg, ei1, edge_attr)
    e2, bt2, T2 = _shard_graph(cfg, ei2)
    wp1, K1 = _degpad(cfg, ei1[1], edge_attr)
    wp2, K2 = _degpad(cfg, ei2[1], np.ones(E, np.float32))
    _log(f"prep done T1={T1} T2={T2} K1={K1} K2={K2}")

    # ---- N1: degrees -> dinv --------------------------------------------
    n1 = build_n1(cfg, K1, K2)
    r1 = SpmdRunner(n1)
    r1.stage([{"wp1": wp1[c], "wp2": wp2[c]} for c in range(NCORES)])
    t0 = time.time(); out1 = r1.run(); _TIMES["n1"] = time.time() - t0
    res1 = r1.results(out1)
    dinv1f = np.concatenate([res1[c]["dinv1"].T.reshape(-1)
                             for c in range(NCORES)])
    dinv2f = np.concatenate([res1[c]["dinv2"].T.reshape(-1)
                             for c in range(NCORES)])
    _DBG["dinv1"], _DBG["dinv2"] = dinv1f, dinv2f
    _log("N1 done")

    # ---- N2: conv1 + W_out ----------------------------------------------
    n2 = build_conv(cfg, bt1, layer=1)
    r2 = SpmdRunner(n2)
    maps2 = []
    W_in16 = np.asarray(W_in, np.float32).astype(np.float16)
    W_out16 = np.asarray(W_out, np.float32).astype(np.float16)
    b_in_l = np.asarray(b_in, np.float32).reshape(2, P).T.copy()
    for c in range(NCORES):
        ec = e1[c]
        coef = np.where(ec["valid"],
                        dinv1f[ec["rows"]] * ec["wv"] * dinv1f[ec["cols"]],
                        0.0).astype(np.float32)
        maps2.append({
            "m1": _feat_stream(x_pad, ec["rows"], coef, T1, D),
            "st8": _sel_stream(ec["colrel"], ec["valid"], T1),
            "w_in": W_in16, "b_in": b_in_l, "w_out": W_out16,
        })
    r2.stage(maps2)
    _log("N2 staged")
    t0 = time.time(); out2 = r2.run(); _TIMES["n2"] = time.time() - t0
    res2 = r2.results(out2)
    for c in range(NCORES):
        e1[c]["h2s"] = res2[c]["h2s"]
    h2T = _unperm(cfg, e1, "h2s")            # [128, NPAD] fp16
    _DBG["h2T"] = h2T
    _log("N2 done")

    # ---- N3: conv2 -> u, v ----------------------------------------------
    n3 = build_conv(cfg, bt2, layer=2)
    r3 = SpmdRunner(n3)
    h2tbl = np.ascontiguousarray(h2T.T)      # [NPAD, 128] fp16
    W1_16 = np.asarray(W1, np.float32).astype(np.float16)
    maps3 = []
    for c in range(NCORES):
        ec = e2[c]
        coef = np.where(ec["valid"],
                        dinv2f[ec["rows"]] * dinv2f[ec["cols"]],
                        0.0).astype(np.float32)
        maps3.append({
            "m1": _feat_stream(h2tbl, ec["rows"], coef, T2, D),
            "st8": _sel_stream(ec["colrel"], ec["valid"], T2),
            "w1": W1_16,
            "b1v": np.asarray(b1, np.float32)[:, None],
            "b_out": np.asarray(b_out, np.float32)[:, None],
        })
    r3.stage(maps3)
    _log("N3 staged")
    t0 = time.time(); out3 = r3.run(); _TIMES["n3"] = time.time() - t0
    res3 = r3.results(out3)
    for c in range(NCORES):
        e2[c]["us"] = res3[c]["us"]
        e2[c]["vs"] = res3[c]["vs"]
    uT = _unperm(cfg, e2, "us")
    vT = _unperm(cfg, e2, "vs")
    _DBG["uT"], _DBG["vT"] = uT, vT
    _log("N3 done")

    # ---- N4: edge MLP decoder -------------------------------------------
    S4 = max(((len(e1[c]["sel"]) + N4_BLK - 1) // N4_BLK) * N4_BLK
             for c in range(NCORES))
    n4 = build_n4(cfg, S4)
    r4 = SpmdRunner(n4)
    row1, col1 = ei1[0], ei1[1]
    W3_16 = np.asarray(W3, np.float32).astype(np.float16)   # [D, 1]
    w3z = np.zeros((P, 2 * N4_CB), np.float16)
    w3z[:, N4_CB] = W3_16[:, 0]
    maps4 = []
    for c in range(NCORES):
        sel = e1[c]["sel"]
        gu = np.zeros((P, S4), np.float16)
        gv = np.zeros((P, S4), np.float16)
        gu[:, :len(sel)] = uT[:, row1[sel]]
        gv[:, :len(sel)] = vT[:, col1[sel]]
        nb4 = S4 // N4_BLK
        guv = np.empty((P, 2 * S4), np.float16)
        gr = guv.reshape(P, nb4, 2, N4_BLK)
        gr[:, :, 0, :] = gu.reshape(P, nb4, N4_BLK)
        gr[:, :, 1, :] = gv.reshape(P, nb4, N4_BLK)
        maps4.append({
            "guv": guv,
            "w2": np.asarray(W2, np.float32).astype(np.float16),
            "b2v": np.asarray(b2, np.float32)[:, None],
            "w3z": w3z,
            "b3v": np.full((N4_CB, 1), np.float32(np.asarray(b3)[0])),
        })
    r4.stage(maps4)
    _log("N4 staged")
    t0 = time.time(); out4 = r4.run(); _TIMES["n4"] = time.time() - t0
    res4 = r4.results(out4)
    _log("N4 done")

    # ---- unshard ---------------------------------------------------------
    # oute[c, g*CH+col] = edge g*BLK + c*CH + col
    result = np.zeros(E, np.float32)
    for c in range(NCORES):
        sel = e1[c]["sel"]
        ot = res4[c]["oute"].astype(np.float32)      # [CB, NBLK*CH]
        ngrp = S4 // N4_BLK
        flat = ot.reshape(N4_CB, ngrp, N4_CH).transpose(1, 0, 2).reshape(-1)
        result[sel] = flat[:len(sel)]
    _TIMES["total_wall"] = time.time() - t_all
    _DBG["runners"] = {"n1": r1, "n2": r2, "n3": r3, "n4": r4}
    _DBG["builders"] = {
        "n1": lambda R: build_n1(cfg, K1, K2, repeat=R),
        "n2": lambda R: build_conv(cfg, bt1, layer=1, repeat=R),
        "n3": lambda R: build_conv(cfg, bt2, layer=2, repeat=R),
        "n4": lambda R: build_n4(cfg, S4, repeat=R),
    }
    return result


# revision 22
# speedup vs baseline: 5.3945x; 5.3945x over previous
"""AlternateGCN on 8 TRN2 NeuronCores.

Strategy (edge/data parallel, dest-sharded):
  - Edges sharded by DESTINATION-node owner core (8 node shards of 6272),
    sorted by destination. Segment-sum runs ON DEVICE as dense PE matmuls
    in TRANSPOSED orientation: per 128-edge tile, Y^T[d, n] += M^T @ St
    accumulates in PSUM, where M = per-edge message tile (fp16 stream,
    GCN normalization c_e = dinv[row]*w*dinv[col] pre-folded during the
    host-side gather, like PyG's cached gcn_norm) and St[e, n] is an EXACT
    0/1 selection matrix streamed in fp8e4 (halves its DMA bytes; 0/1 are
    exact in fp8). No per-tile DVE build ops -> no DVE-sequencer
    bottleneck; conv NEFFs are DMA-roofline bound.
  - ELU computed as elu(z)+1 = relu(z) + exp(-relu(-z)); the "-1" is
    folded into the NEXT layer's bias; 3 ACT ops + 1 DVE op per chunk.
  - Degree -> dinv still computed on device (N1); host only gathers
    dinv[row]*w*dinv[col] into the message stream during shard/sort prep.
  - Decoder (N4): streams u[src], v[dst] as fp16 [feat, edge] streams;
    u+v is computed BY THE DMA (SWDGE accumulate-DMA of the v stream onto
    the u tile), relu on DVE, W2 matmul on PE, relu+bias alternating
    ACT/DVE, and the W3 dot-product uses a sliding-window diagonal
    stationary (w3 placed at a chunk-dependent column of a zero tile) so
    16 chunks' scalar outputs land on 16 distinct PSUM partitions and are
    copied out in ONE [16, 512] op -- no single-partition copies.
  - 4 NEFF stages: N1 degrees->dinv; N2 layer-1 conv -> h2 (pre-multiplied
    by W_out); N3 layer-2 conv -> uT,vT decoder tables; N4 edge MLP.

Self-contained; hardcodes problem shapes (N=50000, E=800000, D=128, H=256).
"""
import sys, time
sys.path.insert(0, "/opt/trn_rl_repo")
import numpy as np
import jax
from jax.sharding import Mesh, PartitionSpec
from jax.experimental.shard_map import shard_map

import concourse.bacc as bacc
import concourse.bass as bass
import concourse.mybir as mybir
import concourse.tile as tile
from concourse import bass2jax
from concourse.bass2jax import _bass_exec_p, install_neuronx_cc_hook

P = 128
NCORES = 8
AF = mybir.ActivationFunctionType
ALU = mybir.AluOpType
DT = mybir.dt
F8 = DT.float8e4
F16 = DT.float16
F32 = DT.float32
NPF8 = DT.np(F8)
VERBOSE = bool(int(__import__("os").environ.get("KERNEL_VERBOSE", "0")))


def _log(msg):
    if VERBOSE:
        print(f"[kernel {time.strftime('%H:%M:%S')}] {msg}", flush=True)


# ---------------------------------------------------------------- runner ----
class SpmdRunner:
    def __init__(self, nc, n_cores=NCORES, donate=False):
        install_neuronx_cc_hook()
        self.nc, self.n_cores = nc, n_cores
        self.donate = donate
        pname = nc.partition_id_tensor.name if nc.partition_id_tensor else None
        in_names, out_names, out_avals, zero_outs = [], [], [], []
        for alloc in nc.m.functions[0].allocations:
            if not isinstance(alloc, mybir.MemoryLocationSet):
                continue
            name = alloc.memorylocations[0].name
            if alloc.kind == "ExternalInput":
                if name != pname:
                    in_names.append(name)
            elif alloc.kind == "ExternalOutput":
                out_names.append(name)
                shape = tuple(alloc.tensor_shape)
                dtype = DT.np(alloc.dtype)
                out_avals.append(jax.core.ShapedArray(shape, dtype))
                zero_outs.append(np.zeros(shape, dtype))
        self.in_names, self.out_names = in_names, out_names
        self.out_avals, self.zero_outs = out_avals, zero_outs
        self.n_params, self.n_outs = len(in_names), len(out_avals)
        all_in = list(in_names) + list(out_names)
        if pname is not None:
            all_in.append(pname)

        def _body(*args):
            operands = list(args)
            if pname is not None:
                operands.append(bass2jax.partition_id_tensor())
            return tuple(_bass_exec_p.bind(
                *operands, out_avals=tuple(out_avals), in_names=tuple(all_in),
                out_names=tuple(out_names), lowering_input_output_aliases=(),
                sim_require_finite=True, sim_require_nnan=True, nc=nc))

        devices = jax.devices()[:n_cores]
        mesh = Mesh(np.asarray(devices), ("core",))
        donate_idx = tuple(range(self.n_params, self.n_params + self.n_outs)) \
            if self.donate else ()
        self.fn = jax.jit(
            shard_map(_body, mesh=mesh,
                      in_specs=(PartitionSpec("core"),) * (self.n_params + self.n_outs),
                      out_specs=(PartitionSpec("core"),) * len(out_names)),
            donate_argnums=donate_idx, keep_unused=True)
        self.dev_inputs = None
        self.dev_zeros = None

    def stage(self, in_maps):
        concat = [np.concatenate([np.ascontiguousarray(in_maps[c][n])
                                  for c in range(self.n_cores)], axis=0)
                  for n in self.in_names]
        self.dev_inputs = [jax.device_put(a) for a in concat]
        jax.block_until_ready(self.dev_inputs)

    def run(self):
        if self.donate:
            zeros = [np.zeros((self.n_cores * z.shape[0], *z.shape[1:]), z.dtype)
                     for z in self.zero_outs]
        else:
            if self.dev_zeros is None:
                self.dev_zeros = [jax.device_put(
                    np.zeros((self.n_cores * z.shape[0], *z.shape[1:]), z.dtype))
                    for z in self.zero_outs]
                jax.block_until_ready(self.dev_zeros)
            zeros = self.dev_zeros
        out = self.fn(*self.dev_inputs, *zeros)
        jax.block_until_ready(out)
        return out

    def results(self, out_arrs):
        return [
            {n: np.asarray(out_arrs[i]).reshape(self.n_cores, *self.out_avals[i].shape)[c]
             for i, n in enumerate(self.out_names)}
            for c in range(self.n_cores)]


# ------------------------------------------------------------- geometry ----
class Cfg:
    def __init__(self, N=50000, D=128, H=256):
        self.N, self.D, self.H = N, D, H
        self.NPAD = ((N + NCORES * P - 1) // (NCORES * P)) * (NCORES * P)
        self.SHARD = self.NPAD // NCORES
        self.WPC = self.SHARD // P           # real windows per core (49)
        self.WGL = self.NPAD // P            # global windows
        self.BW = 4                          # windows per epilogue block
        self.WSLOT = ((self.WPC + self.BW - 1) // self.BW) * self.BW  # 52


# ------------------------------------------------------- device helpers ----
def _rsqrt_masked(nc, out_sb, deg_sb, pool, shape):
    """out = 1/sqrt(deg) where deg>0 else 0 (no Rsqrt, no inf*0)."""
    m = pool.tile(shape, F32, tag="rq_m")
    t = pool.tile(shape, F32, tag="rq_t")
    nc.vector.tensor_scalar(out=m[:], in0=deg_sb, scalar1=0.0, scalar2=None,
                            op0=ALU.is_gt)
    nc.scalar.activation(out=t[:], in_=deg_sb, func=AF.Sqrt)
    nc.vector.tensor_scalar(out=t[:], in0=t[:], scalar1=1.0, scalar2=None,
                            op0=ALU.add)
    nc.vector.tensor_tensor(out=t[:], in0=t[:], in1=m[:], op=ALU.subtract)
    nc.vector.reciprocal(out=out_sb, in_=t[:])
    nc.vector.tensor_tensor(out=out_sb, in0=out_sb, in1=m[:], op=ALU.mult)


# ------------------------------------------------------------------ N1 -----
def build_n1(cfg, K1, K2, repeat=1):
    """Per-shard degree reduce for both graphs -> dinv1, dinv2 [128, WPC]."""
    nc = bacc.Bacc("TRN2", target_bir_lowering=False)
    W = cfg.WPC
    wp1 = nc.dram_tensor("wp1", [P, W * K1], F32, kind="ExternalInput")
    wp2 = nc.dram_tensor("wp2", [P, W * K2], F32, kind="ExternalInput")
    dinv1 = nc.dram_tensor("dinv1", [P, W], F32, kind="ExternalOutput")
    dinv2 = nc.dram_tensor("dinv2", [P, W], F32, kind="ExternalOutput")
    with tile.TileContext(nc) as tc:
        with tc.tile_pool(name="p", bufs=1) as pool:
            for _ in range(repeat):
                for tag, wp, K, dout in (("a", wp1, K1, dinv1),
                                         ("b", wp2, K2, dinv2)):
                    wt = pool.tile([P, W, K], F32, tag=f"wt{tag}")
                    nc.sync.dma_start(out=wt[:].rearrange("p a b -> p (a b)"),
                                      in_=wp[:, :])
                    deg = pool.tile([P, W], F32, tag=f"deg{tag}")
                    nc.vector.tensor_reduce(out=deg[:, :, None], in_=wt[:],
                                            axis=mybir.AxisListType.X, op=ALU.add)
                    o = pool.tile([P, W], F32, tag=f"o{tag}")
                    _rsqrt_masked(nc, o[:], deg[:], pool, [P, W])
                    nc.sync.dma_start(out=dout[:, :], in_=o[:])
    nc.finalize()
    return nc


# ----------------------------------------------------------- N2 and N3 -----
def build_conv(cfg, bt, layer, repeat=1):
    """Conv layer, transposed pipeline.

    bt: per-slot tile budgets [WSLOT]. T = sum(bt).
    Streams: m1 [P, T*D] fp16 = c_e * x[row_e] messages; st8 [P, T*P]
    fp8e4 EXACT 0/1 selection (st8[e, n] = 1 iff col_rel_e == n).
    Per tile: one matmul Y^T += m1_tile^T @ st_tile.
    layer=1: x1T = elu(W_in^T @ Y1T + b_in) -> h2T = W_out^T @ x1T -> h2s.
    layer=2: x2T = elu(Y2T + b_out) -> uT = W1a^T x2T + b1, vT = W1b^T x2T.
    """
    nc = bacc.Bacc("TRN2", target_bir_lowering=False)
    D, H, BW, WSLOT = cfg.D, cfg.H, cfg.BW, cfg.WSLOT
    T = int(np.sum(bt))
    to = np.zeros(WSLOT + 1, np.int64)
    np.cumsum(bt, out=to[1:])
    BTMAX = int(np.max(bt))
    NBLK = WSLOT // BW
    NB = BW * P                              # block node columns (512)

    m1 = nc.dram_tensor("m1", [P, T * D], F16, kind="ExternalInput")
    st8 = nc.dram_tensor("st8", [P, T * P], F8, kind="ExternalInput")
    if layer == 1:
        w_in = nc.dram_tensor("w_in", [D, H], F16, kind="ExternalInput")
        b_in = nc.dram_tensor("b_in", [P, 2], F32, kind="ExternalInput")
        w_out = nc.dram_tensor("w_out", [H, D], F16, kind="ExternalInput")
        h2s = nc.dram_tensor("h2s", [P, WSLOT * P], F16, kind="ExternalOutput")
    else:
        w1 = nc.dram_tensor("w1", [2 * D, D], F16, kind="ExternalInput")
        b1v = nc.dram_tensor("b1v", [P, 1], F32, kind="ExternalInput")
        b_out = nc.dram_tensor("b_out", [P, 1], F32, kind="ExternalInput")
        us = nc.dram_tensor("us", [P, WSLOT * P], F16, kind="ExternalOutput")
        vs = nc.dram_tensor("vs", [P, WSLOT * P], F16, kind="ExternalOutput")

    with tile.TileContext(nc) as tc:
        with tc.tile_pool(name="const", bufs=1) as cst:
            if layer == 1:
                w_in_sb = cst.tile([P, H], F16, tag="w_in")
                nc.sync.dma_start(out=w_in_sb[:], in_=w_in[:, :])
                wo_a = cst.tile([P, D], F16, tag="wo_a")
                wo_b = cst.tile([P, D], F16, tag="wo_b")
                nc.sync.dma_start(out=wo_a[:], in_=w_out[0:P, :])
                nc.sync.dma_start(out=wo_b[:], in_=w_out[P:2 * P, :])
                b_in_sb = cst.tile([P, 2], F32, tag="b_in")
                nc.sync.dma_start(out=b_in_sb[:], in_=b_in[:, :])
                nb_in = cst.tile([P, 2], F32, tag="nb_in")
                nc.vector.tensor_scalar(out=nb_in[:], in0=b_in_sb[:],
                                        scalar1=-1.0, scalar2=None, op0=ALU.mult)
            else:
                w1a = cst.tile([P, D], F16, tag="w1a")
                w1b = cst.tile([P, D], F16, tag="w1b")
                nc.sync.dma_start(out=w1a[:], in_=w1[0:P, :])
                nc.sync.dma_start(out=w1b[:], in_=w1[P:2 * P, :])
                b1_sb = cst.tile([P, 1], F32, tag="b1")
                nc.sync.dma_start(out=b1_sb[:], in_=b1v[:, :])
                bo_sb = cst.tile([P, 1], F32, tag="bo")
                nc.sync.dma_start(out=bo_sb[:], in_=b_out[:, :])
                nbo_sb = cst.tile([P, 1], F32, tag="nbo")
                nc.vector.tensor_scalar(out=nbo_sb[:], in0=bo_sb[:],
                                        scalar1=-1.0, scalar2=None, op0=ALU.mult)
            _conv_main(nc, tc, cfg, bt, layer, repeat, locals())
    nc.finalize()
    return nc


def _conv_main(nc, tc, cfg, bt, layer, repeat, env):
    D, H, BW, WSLOT = cfg.D, cfg.H, cfg.BW, cfg.WSLOT
    to = np.zeros(WSLOT + 1, np.int64)
    np.cumsum(bt, out=to[1:])
    BTMAX = int(np.max(bt))
    NBLK = WSLOT // BW
    NB = BW * P
    m1, st8 = env["m1"], env["st8"]
    if layer == 1:
        w_in_sb, wo_a, wo_b = env["w_in_sb"], env["wo_a"], env["wo_b"]
        b_in_sb, nb_in = env["b_in_sb"], env["nb_in"]
        h2s = env["h2s"]
    else:
        w1a, w1b = env["w1a"], env["w1b"]
        bo_sb, nbo_sb = env["bo_sb"], env["nbo_sb"]
        b1_sb = env["b1_sb"]
        us, vs = env["us"], env["vs"]
    # DMA super-blocks of 8 windows (~6 MB transfers) for near-roofline DMA;
    # epilogue still runs per 4-window block.
    SBW = 2 * BW
    NSUP = (WSLOT + SBW - 1) // SBW
    sup_lo = [b * SBW for b in range(NSUP)]
    sup_hi = [min(WSLOT, b * SBW + SBW) for b in range(NSUP)]
    MSUP = int(max(to[h] - to[l] for l, h in zip(sup_lo, sup_hi)))
    with (tc.tile_pool(name="mt", bufs=3) as mtp,
          tc.tile_pool(name="stt", bufs=3) as stp,
          tc.tile_pool(name="ypsum", bufs=3, space="PSUM") as yp,
          tc.tile_pool(name="xpsum", bufs=2, space="PSUM") as xp,
          tc.tile_pool(name="hpsum", bufs=2, space="PSUM") as hp,
          tc.tile_pool(name="slab", bufs=2) as slp,
          tc.tile_pool(name="fl", bufs=2) as fl):
            for _ in range(repeat):
                for blk in range(NBLK):
                    sup = (blk * BW) // SBW
                    if blk * BW == sup_lo[sup]:
                        b0 = int(to[sup_lo[sup]])
                        nb = int(to[sup_hi[sup]]) - b0
                        mw = mtp.tile([P, MSUP * D], F16, tag="mw")
                        nc.sync.dma_start(out=mw[:, 0:nb * D],
                                          in_=m1[:, b0 * D:(b0 + nb) * D])
                        sw = stp.tile([P, MSUP * P], F8, tag="sw")
                        nc.scalar.dma_start(out=sw[:, 0:nb * P],
                                            in_=st8[:, b0 * P:(b0 + nb) * P])
                    ysl = slp.tile([P, NB], F16, tag="ysl")
                    for j in range(BW):
                        k = blk * BW + j
                        nt = int(bt[k])
                        lo = int(to[k]) - b0
                        y = yp.tile([P, P], F32, tag="y")
                        for s in range(nt):
                            t = lo + s
                            nc.tensor.matmul(out=y[:],
                                             lhsT=mw[:, t * D:(t + 1) * D],
                                             rhs=sw[:, t * P:(t + 1) * P],
                                             start=(s == 0), stop=(s == nt - 1))
                        # PSUM -> SBUF copy on DVE (ACT is epilogue-bound)
                        nc.vector.tensor_scalar(out=ysl[:, j * P:(j + 1) * P],
                                                in0=y[:], scalar1=0.0,
                                                scalar2=None, op0=ALU.add)
                    if layer == 1:
                        x1c = []
                        for hh in range(2):
                            x1p = xp.tile([P, NB], F32, tag="x1p")
                            nc.tensor.matmul(
                                out=x1p[:],
                                lhsT=w_in_sb[:, hh * P:(hh + 1) * P],
                                rhs=ysl[:], start=True, stop=True)
                            # elu(z) = (relu(z) - 1) + exp(-relu(-z));
                            # a/e kept f32 so the near-1 cancellation is exact
                            a = fl.tile([P, NB], F32, tag=f"a{hh}")
                            nc.vector.tensor_scalar(
                                out=a[:], in0=x1p[:],
                                scalar1=b_in_sb[:, hh:hh + 1], scalar2=0.0,
                                op0=ALU.add, op1=ALU.max)
                            rn = fl.tile([P, NB], F16, tag=f"rn{hh}")
                            nc.scalar.activation(out=rn[:], in_=x1p[:],
                                                 func=AF.Relu, scale=-1.0,
                                                 bias=nb_in[:, hh:hh + 1])
                            e = fl.tile([P, NB], F32, tag=f"e{hh}")
                            nc.scalar.activation(out=e[:], in_=rn[:],
                                                 func=AF.Exp, scale=-1.0)
                            x1 = fl.tile([P, NB], F16, tag=f"x1{hh}")
                            nc.vector.scalar_tensor_tensor(
                                out=x1[:], in0=a[:], scalar=-1.0, in1=e[:],
                                op0=ALU.add, op1=ALU.add)
                            x1c.append(x1)
                        h2p = hp.tile([P, NB], F32, tag="h2p")
                        nc.tensor.matmul(out=h2p[:], lhsT=wo_a[:],
                                         rhs=x1c[0][:], start=True, stop=False)
                        nc.tensor.matmul(out=h2p[:], lhsT=wo_b[:],
                                         rhs=x1c[1][:], start=False, stop=True)
                        hsb = fl.tile([P, NB], F16, tag="hsb")
                        nc.scalar.activation(out=hsb[:], in_=h2p[:],
                                             func=AF.Identity)
                        nc.sync.dma_start(
                            out=h2s[:, blk * NB:(blk + 1) * NB], in_=hsb[:])
                    else:
                        # x2 = elu(ysl + b_out), same f32-cancellation scheme
                        a = fl.tile([P, NB], F32, tag="a2")
                        nc.scalar.activation(out=a[:], in_=ysl[:],
                                             func=AF.Relu, bias=bo_sb[:, 0:1])
                        rn = fl.tile([P, NB], F16, tag="rn2")
                        nc.scalar.activation(out=rn[:], in_=ysl[:],
                                             func=AF.Relu, scale=-1.0,
                                             bias=nbo_sb[:, 0:1])
                        e = fl.tile([P, NB], F32, tag="e2")
                        nc.scalar.activation(out=e[:], in_=rn[:],
                                             func=AF.Exp, scale=-1.0)
                        x2 = fl.tile([P, NB], F16, tag="x2")
                        nc.vector.scalar_tensor_tensor(
                            out=x2[:], in0=a[:], scalar=-1.0, in1=e[:],
                            op0=ALU.add, op1=ALU.add)
                        up = xp.tile([P, NB], F32, tag="up")
                        nc.tensor.matmul(out=up[:], lhsT=w1a[:], rhs=x2[:],
                                         start=True, stop=True)
                        u_sb = fl.tile([P, NB], F16, tag="u_sb")
                        nc.scalar.activation(out=u_sb[:], in_=up[:],
                                             func=AF.Identity,
                                             bias=b1_sb[:, 0:1])
                        nc.sync.dma_start(
                            out=us[:, blk * NB:(blk + 1) * NB], in_=u_sb[:])
                        vp = hp.tile([P, NB], F32, tag="vp")
                        nc.tensor.matmul(out=vp[:], lhsT=w1b[:], rhs=x2[:],
                                         start=True, stop=True)
                        v_sb = fl.tile([P, NB], F16, tag="v_sb")
                        nc.scalar.activation(out=v_sb[:], in_=vp[:],
                                             func=AF.Identity)
                        nc.sync.dma_start(
                            out=vs[:, blk * NB:(blk + 1) * NB], in_=v_sb[:])


# ------------------------------------------------------------------ N4 -----
N4_CH = 512                                  # edges per chunk (1 PSUM bank)
N4_CB = 16                                   # chunks per block / out rows
N4_BLK = N4_CH * N4_CB                       # 8192 edges per block


def build_n4(cfg, S4, repeat=1):
    """Decoder: out = relu(relu(u[src]+v[dst]) @ W2 + b2) @ W3 + b3.

    guv: single host-interleaved fp16 stream ([u-block | v-block] per
    8192-edge block -> one 4 MB DMA per block). Per 1024-edge sub-chunk:
    u+v and relu on DVE, two W2 matmuls into a 2-bank PSUM tile, one
    relu+bias on ACT, and W3 matmuls whose stationary is a sliding window
    of a zero tile with w3 at column N4_CB, so chunk c's [1, 512] result
    lands on PSUM partition c; one [16, 512] bias-copy per block drains
    all 16 chunks. Output: oute[c, g*512+col] = edge g*8192+c*512+col.
    (A SWDGE accumulate-DMA for u+v desyncs the axon mesh - done on DVE.)
    """
    nc = bacc.Bacc("TRN2", target_bir_lowering=False)
    D = cfg.D
    CH, CB, BLK = N4_CH, N4_CB, N4_BLK
    NBLK = S4 // BLK
    assert S4 % BLK == 0
    guv = nc.dram_tensor("guv", [P, 2 * S4], F16, kind="ExternalInput")
    w2 = nc.dram_tensor("w2", [D, D], F16, kind="ExternalInput")
    b2v = nc.dram_tensor("b2v", [P, 1], F32, kind="ExternalInput")
    w3z = nc.dram_tensor("w3z", [P, 2 * CB], F16, kind="ExternalInput")
    b3v = nc.dram_tensor("b3v", [CB, 1], F32, kind="ExternalInput")
    oute = nc.dram_tensor("oute", [CB, NBLK * CH], F16, kind="ExternalOutput")
    with tile.TileContext(nc) as tc:
        with (tc.tile_pool(name="const", bufs=1) as cst,
              tc.tile_pool(name="guv", bufs=4) as gup,
              tc.tile_pool(name="h1r", bufs=4) as hrp,
              tc.tile_pool(name="h2ps", bufs=3, space="PSUM") as h2ps,
              tc.tile_pool(name="ops", bufs=2, space="PSUM") as ops,
              tc.tile_pool(name="sb", bufs=6) as sbp,
              tc.tile_pool(name="ob", bufs=2) as obp):
            w2_sb = cst.tile([P, D], F16, tag="w2")
            nc.sync.dma_start(out=w2_sb[:], in_=w2[:, :])
            w3z_sb = cst.tile([P, 2 * CB], F16, tag="w3z")
            nc.sync.dma_start(out=w3z_sb[:], in_=w3z[:, :])
            b2_sb = cst.tile([P, 1], F32, tag="b2")
            nc.sync.dma_start(out=b2_sb[:], in_=b2v[:, :])
            b3_sb = cst.tile([CB, 1], F32, tag="b3")
            nc.sync.dma_start(out=b3_sb[:], in_=b3v[:, :])
            for _ in range(repeat):
                for blk in range(NBLK):
                    guvt = gup.tile([P, 2 * BLK], F16, tag="guv")
                    nc.sync.dma_start(out=guvt[:],
                                      in_=guv[:, blk * 2 * BLK:(blk + 1) * 2 * BLK])
                    op = ops.tile([CB, CH], F32, tag="op")
                    for c2 in range(CB // 2):
                        # 1024-edge sub-chunks: halves ACT/DVE op overheads
                        dsl = slice(c2 * 2 * CH, (c2 + 1) * 2 * CH)
                        vsl = slice(BLK + c2 * 2 * CH, BLK + (c2 + 1) * 2 * CH)
                        h1a = hrp.tile([P, 2 * CH], F16, tag="h1a")
                        nc.vector.tensor_tensor(out=h1a[:], in0=guvt[:, dsl],
                                                in1=guvt[:, vsl], op=ALU.add)
                        h1r = hrp.tile([P, 2 * CH], F16, tag="h1r")
                        nc.vector.tensor_scalar(out=h1r[:], in0=h1a[:],
                                                scalar1=0.0, scalar2=None,
                                                op0=ALU.max)
                        h2 = h2ps.tile([P, 2 * CH], F32, tag="h2")
                        nc.tensor.matmul(out=h2[:, 0:CH], lhsT=w2_sb[:],
                                         rhs=h1r[:, 0:CH],
                                         start=True, stop=True)
                        nc.tensor.matmul(out=h2[:, CH:2 * CH], lhsT=w2_sb[:],
                                         rhs=h1r[:, CH:2 * CH],
                                         start=True, stop=True)
                        h2r = sbp.tile([P, 2 * CH], F16, tag="h2r")
                        nc.scalar.activation(out=h2r[:], in_=h2[:],
                                             func=AF.Relu,
                                             bias=b2_sb[:, 0:1])
                        for ci in range(2):
                            c = 2 * c2 + ci
                            # chunk c's dot-product lands on PSUM row c:
                            # lhsT = w3z[:, CB-c : 2*CB-c] has w3 at column c
                            nc.tensor.matmul(
                                out=op[:],
                                lhsT=w3z_sb[:, CB - c:2 * CB - c],
                                rhs=h2r[:, ci * CH:(ci + 1) * CH],
                                start=(c == 0), stop=(c == CB - 1))
                    otb = obp.tile([CB, CH], F16, tag="otb")
                    nc.scalar.activation(out=otb[:], in_=op[:],
                                         func=AF.Identity, bias=b3_sb[:, 0:1])
                    nc.sync.dma_start(
                        out=oute[:, blk * CH:(blk + 1) * CH], in_=otb[:])
    nc.finalize()
    return nc


# ------------------------------------------------------------ host prep ----
def _shard_graph(cfg, ei, w=None):
    """Shard edges by col-owner core, sort by col, assign windows to slots
    sorted by edge count so a shared per-slot budget has little padding."""
    row, col = np.asarray(ei[0]), np.asarray(ei[1])
    owner = col // cfg.SHARD
    cores = []
    for c in range(NCORES):
        sel = np.where(owner == c)[0]
        o = np.argsort(col[sel], kind="stable")
        sel = sel[o]
        colc = col[sel]
        win = (colc - c * cfg.SHARD) // P
        cnt = np.bincount(win, minlength=cfg.WPC)
        perm = np.argsort(-cnt, kind="stable")
        cores.append((sel, colc, win, cnt, perm))
    tiles = np.ones((NCORES, cfg.WSLOT), np.int64)
    for c in range(NCORES):
        _, _, _, cnt, perm = cores[c]
        tiles[c, :cfg.WPC] = np.maximum((cnt[perm] + P - 1) // P, 1)
    bt = tiles.max(axis=0)
    to = np.zeros(cfg.WSLOT + 1, np.int64)
    np.cumsum(bt, out=to[1:])
    T = int(to[-1])
    S = T * P
    out = []
    for c in range(NCORES):
        sel, colc, win, cnt, perm = cores[c]
        offs = np.zeros(cfg.WPC + 1, np.int64)
        np.cumsum(cnt, out=offs[1:])
        pos = np.arange(len(sel)) - offs[win]
        islot = np.empty(cfg.WPC, np.int64)
        islot[perm] = np.arange(cfg.WPC)
        slot = to[islot[win]] * P + pos
        rows_s = np.zeros(S, np.int64)
        cols_s = np.zeros(S, np.int64)
        colrel = np.zeros(S, np.int64)
        valid = np.zeros(S, bool)
        rows_s[slot] = row[sel]
        cols_s[slot] = colc
        colrel[slot] = colc % P
        valid[slot] = True
        d = dict(sel=sel, slot=slot, rows=rows_s, cols=cols_s, colrel=colrel,
                 valid=valid, perm=perm)
        if w is not None:
            ws = np.zeros(S, np.float32)
            ws[slot] = w[sel]
            d["wv"] = ws
        out.append(d)
    return out, bt, T


def _feat_stream(tbl, rows, coef, T, D):
    """[T*128] row ids + per-edge coefficients -> [128, T*D] fp16 message
    stream (coefficient-scaled gather)."""
    g = tbl[rows.reshape(T, P)].astype(np.float32)          # [T, 128, D]
    g *= coef.reshape(T, P)[:, :, None]
    return np.ascontiguousarray(
        g.transpose(1, 0, 2).reshape(P, T * D)).astype(np.float16)


def _sel_stream(colrel, valid, T):
    """[T*128] col_rel + valid -> [128, T*128] fp8 EXACT 0/1 selection."""
    st = np.zeros((P, T * P), NPF8)
    slot = np.where(valid)[0]
    p = slot % P
    t = slot // P
    st[p, t * P + colrel[slot]] = 1.0
    return st


def _unperm(cfg, shards, key):
    """Reassemble [128, NPAD] canonical node-major-T table from per-core
    permuted slot outputs [128, WSLOT*128]."""
    full = np.zeros((P, cfg.NPAD), np.float16)
    cols = np.arange(P)
    for c in range(NCORES):
        perm = shards[c]["perm"]
        src = shards[c][key]
        idx = (c * cfg.SHARD + (perm[:, None] * P + cols[None, :])).reshape(-1)
        full[:, idx] = src[:, :cfg.WPC * P]
    return full


def _degpad(cfg, col, w):
    """Per-core [128, WPC*K] padded weight layout for on-device degree sum."""
    o = np.argsort(col, kind="stable")
    cs, ws = col[o], w[o]
    cnt = np.bincount(cs, minlength=cfg.NPAD)
    K = int(max(cnt.max(), 1))
    offs = np.zeros(cfg.NPAD + 1, np.int64)
    np.cumsum(cnt, out=offs[1:])
    slotn = np.arange(len(cs)) - offs[cs]
    wpad = np.zeros((cfg.NPAD, K), np.float32)
    wpad[cs, slotn] = ws
    per_core = []
    for c in range(NCORES):
        blockc = wpad[c * cfg.SHARD:(c + 1) * cfg.SHARD]
        per_core.append(np.ascontiguousarray(
            blockc.reshape(cfg.WPC, P, K).transpose(1, 0, 2)
            .reshape(P, cfg.WPC * K)))
    return per_core, K


# --------------------------------------------------------------- kernel ----
_TIMES = {}
_DBG = {}


def kernel(node_ids, edge_index, neighbour_edge_index, edge_attr,
           emb, W_in, b_in, W_out, b_out, W1, b1, W2, b2, W3, b3):
    cfg = Cfg()
    D, H = cfg.D, cfg.H
    t_all = time.time()
    node_ids = np.asarray(node_ids)
    emb = np.asarray(emb, np.float32)
    edge_attr = np.asarray(edge_attr, np.float32)
    ei1 = np.asarray(edge_index)
    ei2 = np.asarray(neighbour_edge_index)
    E = ei1.shape[1]
    x_pad = np.zeros((cfg.NPAD, D), np.float32)
    x_pad[:cfg.N] = emb[node_ids]

    # ---- host index prep -------------------------------------------------
    e1, bt1, T1 = _shard_graph(cfg, ei1, edge_attr)
    e2, bt2, T2 = _shard_graph(cfg, ei2)
    wp1, K1 = _degpad(cfg, ei1[1], edge_attr)
    wp2, K2 = _degpad(cfg, ei2[1], np.ones(E, np.float32))
    _log(f"prep done T1={T1} T2={T2} K1={K1} K2={K2}")

    # ---- N1: degrees -> dinv --------------------------------------------
    n1 = build_n1(cfg, K1, K2)
    r1 = SpmdRunner(n1)
    r1.stage([{"wp1": wp1[c], "wp2": wp2[c]} for c in range(NCORES)])
    t0 = time.time(); out1 = r1.run(); _TIMES["n1"] = time.time() - t0
    res1 = r1.results(out1)
    dinv1f = np.concatenate([res1[c]["dinv1"].T.reshape(-1)
                             for c in range(NCORES)])
    dinv2f = np.concatenate([res1[c]["dinv2"].T.reshape(-1)
                             for c in range(NCORES)])
    _DBG["dinv1"], _DBG["dinv2"] = dinv1f, dinv2f
    _log("N1 done")

    # ---- N2: conv1 + W_out ----------------------------------------------
    n2 = build_conv(cfg, bt1, layer=1)
    r2 = SpmdRunner(n2)
    maps2 = []
    W_in16 = np.asarray(W_in, np.float32).astype(np.float16)
    W_out16 = np.asarray(W_out, np.float32).astype(np.float16)
    b_in_l = np.asarray(b_in, np.float32).reshape(2, P).T.copy()
    for c in range(NCORES):
        ec = e1[c]
        coef = np.where(ec["valid"],
                        dinv1f[ec["rows"]] * ec["wv"] * dinv1f[ec["cols"]],
                        0.0).astype(np.float32)
        maps2.append({
            "m1": _feat_stream(x_pad, ec["rows"], coef, T1, D),
            "st8": _sel_stream(ec["colrel"], ec["valid"], T1),
            "w_in": W_in16, "b_in": b_in_l, "w_out": W_out16,
        })
    r2.stage(maps2)
    _log("N2 staged")
    t0 = time.time(); out2 = r2.run(); _TIMES["n2"] = time.time() - t0
    res2 = r2.results(out2)
    for c in range(NCORES):
        e1[c]["h2s"] = res2[c]["h2s"]
    h2T = _unperm(cfg, e1, "h2s")            # [128, NPAD] fp16
    _DBG["h2T"] = h2T
    _log("N2 done")

    # ---- N3: conv2 -> u, v ----------------------------------------------
    n3 = build_conv(cfg, bt2, layer=2)
    r3 = SpmdRunner(n3)
    h2tbl = np.ascontiguousarray(h2T.T)      # [NPAD, 128] fp16
    W1_16 = np.asarray(W1, np.float32).astype(np.float16)
    maps3 = []
    for c in range(NCORES):
        ec = e2[c]
        coef = np.where(ec["valid"],
                        dinv2f[ec["rows"]] * dinv2f[ec["cols"]],
                        0.0).astype(np.float32)
        maps3.append({
            "m1": _feat_stream(h2tbl, ec["rows"], coef, T2, D),
            "st8": _sel_stream(ec["colrel"], ec["valid"], T2),
            "w1": W1_16,
            "b1v": np.asarray(b1, np.float32)[:, None],
            "b_out": np.asarray(b_out, np.float32)[:, None],
        })
    r3.stage(maps3)
    _log("N3 staged")
    t0 = time.time(); out3 = r3.run(); _TIMES["n3"] = time.time() - t0
    res3 = r3.results(out3)
    for c in range(NCORES):
        e2[c]["us"] = res3[c]["us"]
        e2[c]["vs"] = res3[c]["vs"]
    uT = _unperm(cfg, e2, "us")
    vT = _unperm(cfg, e2, "vs")
    _DBG["uT"], _DBG["vT"] = uT, vT
    _log("N3 done")

    # ---- N4: edge MLP decoder -------------------------------------------
    S4 = max(((len(e1[c]["sel"]) + N4_BLK - 1) // N4_BLK) * N4_BLK
             for c in range(NCORES))
    n4 = build_n4(cfg, S4)
    r4 = SpmdRunner(n4)
    row1, col1 = ei1[0], ei1[1]
    W3_16 = np.asarray(W3, np.float32).astype(np.float16)   # [D, 1]
    w3z = np.zeros((P, 2 * N4_CB), np.float16)
    w3z[:, N4_CB] = W3_16[:, 0]
    maps4 = []
    for c in range(NCORES):
        sel = e1[c]["sel"]
        gu = np.zeros((P, S4), np.float16)
        gv = np.zeros((P, S4), np.float16)
        gu[:, :len(sel)] = uT[:, row1[sel]]
        gv[:, :len(sel)] = vT[:, col1[sel]]
        nb4 = S4 // N4_BLK
        guv = np.empty((P, 2 * S4), np.float16)
        gr = guv.reshape(P, nb4, 2, N4_BLK)
        gr[:, :, 0, :] = gu.reshape(P, nb4, N4_BLK)
        gr[:, :, 1, :] = gv.reshape(P, nb4, N4_BLK)
        maps4.append({
            "guv": guv,
            "w2": np.asarray(W2, np.float32).astype(np.float16),
            "b2v": np.asarray(b2, np.float32)[:, None],
            "w3z": w3z,
            "b3v": np.full((N4_CB, 1), np.float32(np.asarray(b3)[0])),
        })
    r4.stage(maps4)
    _log("N4 staged")
    t0 = time.time(); out4 = r4.run(); _TIMES["n4"] = time.time() - t0
    res4 = r4.results(out4)
    _log("N4 done")

    # ---- unshard ---------------------------------------------------------
    # oute[c, g*CH+col] = edge g*BLK + c*CH + col
    result = np.zeros(E, np.float32)
    for c in range(NCORES):
        sel = e1[c]["sel"]
        ot = res4[c]["oute"].astype(np.float32)      # [CB, NBLK*CH]
        ngrp = S4 // N4_BLK
        flat = ot.reshape(N4_CB, ngrp, N4_CH).transpose(1, 0, 2).reshape(-1)
        result[sel] = flat[:len(sel)]
    _TIMES["total_wall"] = time.time() - t_all
    _DBG["runners"] = {"n1": r1, "n2": r2, "n3": r3, "n4": r4}
    _DBG["builders"] = {
        "n1": lambda R: build_n1(cfg, K1, K2, repeat=R),
        "n2": lambda R: build_conv(cfg, bt1, layer=1, repeat=R),
        "n3": lambda R: build_conv(cfg, bt2, layer=2, repeat=R),
        "n4": lambda R: build_n4(cfg, S4, repeat=R),
    }
    return result
